# revision 22
# baseline (speedup 1.0000x reference)
"""Masked attention on 8 TRN2 NeuronCores.

Full-input contract: kernel(**inputs) takes the complete Q/K/V/mask/dk and
returns the full [32, 2048, 64] output. Internally shards batch 32 -> 4 per
core (data parallel, no communication).

v4 design (per core: 4 batches, S=2048, D=64), scores computed TRANSPOSED
(S^T[k,q] = K @ Q^T) so exp output P^T is already in the layout the P@V
matmul consumes:
  - QK^T: BF16 matmuls, 2-way PE row tiling (chunk c uses row group c%2;
    Q^T/K^T duplicated into both partition halves), chunk-granular score
    tiles (3 PSUM slots) so QK runs 2-3 chunks ahead of exp.
  - exp: chunk pairs split across engines by local pair index p:
      p%4==3 -> DVE one-instruction exp+mask: scalar_tensor_tensor computes
        round(s*A + mb) -> int16, bitcast bf16 ~ exp(s/8)*~mask, where the
        i16 maskbias mb is 16248 (=127*128-8, Schraudolph bias) on kept
        entries and -32400 (saturates/bitcasts to ~-0.0) on masked ones.
      else -> ScalarE activation Exp (scale=1/8) into bf16, then the mask
        multiply: pair-wide on GpSimd (Pool) for p%4==1, per-chunk on DVE
        otherwise (u8 mask expanded to bf16 by one SWDGE casting DMA per
        (b,h), prefetched one h ahead).
  - P@V transposed: stationary is V chunk [128k, 128] whose cols 64:128 are
    all ones -> outT[0:64,q] = (P@V)^T and outT[64:128,q] = row-sum of P
    (softmax denominator) broadcast at zero extra matmul cycles.
  - epilogue per (b,h): one DVE copy of outT[0:65] -> DMA [b, 65, S].
    Host divides rows 0:64 by row 64 and transposes to [B, S, 64] during
    unshard (the on-device DVE reciprocal at this shape costs 6.5us per
    call; the host divide is 4M flops).
  - qt/kt/v2/maskbias loads prefetch one batch / one h ahead on the SP
    queue; u8 mask rides the SWDGE queue.
"""

import sys

import numpy as np

for _p in ("/opt/trn_rl_repo", "/root/.axon_site/_ro/trn_rl_repo"):
    if _p not in sys.path:
        sys.path.append(_p)

import ml_dtypes

import concourse.bacc as bacc
import concourse.bass as bass
import concourse.mybir as mybir
from concourse.bass_utils import run_bass_kernel_spmd
from concourse.tile import TileContext

N_CORES = 8
B, S, D = 32, 2048, 64
BPC = B // N_CORES  # batches per core
NK = S // 128  # 16 k-chunks
NP = NK // 2  # 8 chunk pairs per (b, h)
NH = 2  # q halves
QH = S // NH  # 1024
ND = 2  # DVE-exp pairs per (b, h)  (p % 4 == 3)
VW = 65  # V stationary width: 64 d cols + 1 ones col (denominator)

F32 = mybir.dt.float32
F32R = mybir.dt.float32r
BF16 = mybir.dt.bfloat16
U8 = mybir.dt.uint8
I16 = mybir.dt.int16
EXP = mybir.ActivationFunctionType.Exp
MULT = mybir.AluOpType.mult
ADD = mybir.AluOpType.add

# int16 Schraudolph exp(s/8): bits = round(s*EXP_A + bias); bias for kept
# entries lives in the maskbias tensor (MB_KEEP), masked entries get MB_DROP
# (saturates to 0x8000 = -0.0 in bf16, or ~1e-35 magnitudes).
EXP_A = 128.0 * (np.log2(np.e) / 8.0)
MB_KEEP = 16248  # 127*128 - 8
# drop bias: s*EXP_A + MB_DROP stays in int16 for |s| < 75 (no wrap) and
# bitcasts to negligible bf16 magnitudes (~2^-120)
MB_DROP = -31000

_CACHED_NC = None


def build_nc():
    global _CACHED_NC
    if _CACHED_NC is not None:
        return _CACHED_NC
    nc = bacc.Bacc("TRN2", target_bir_lowering=False)
    QT = nc.dram_tensor("qt", [BPC, 128, S], BF16, kind="ExternalInput")
    KT = nc.dram_tensor("kt", [BPC, 128, S], BF16, kind="ExternalInput")
    # V2[b, p, c*128:(c+1)*128]: cols 0:64 V chunk c, cols 64:128 all 1.0
    V2 = nc.dram_tensor("v2", [BPC, 128, NK * VW], BF16, kind="ExternalInput")
    # NM8[b, h, p, c*1024+q] = ~mask[b, h*1024+q, c*128+p] as u8
    NM8 = nc.dram_tensor("nm8", [BPC, NH, 128, NK * QH], U8, kind="ExternalInput")
    # MB16[b, h, p, (d, ci, q)]: i16 maskbias for DVE pairs (p_local=4d+3)
    MB16 = nc.dram_tensor("mb16", [BPC, NH, 128, ND * 2 * QH], I16, kind="ExternalInput")
    # rows 0:64 unnormalized (P@V)^T, row 64 = softmax denominator
    OUTT = nc.dram_tensor("out_t", [BPC, D + 1, S], F32, kind="ExternalOutput")

    with TileContext(nc) as tc:
        with (
            tc.tile_pool(name="qk", bufs=3) as qk_pool,
            tc.tile_pool(name="vp", bufs=3) as v_pool,
            tc.tile_pool(name="nm", bufs=3) as nm_pool,
            tc.tile_pool(name="mb", bufs=3) as mb_pool,
            tc.tile_pool(name="pt", bufs=7) as pt_pool,
            tc.tile_pool(name="sc", bufs=3, space="PSUM") as sc_pool,
            tc.tile_pool(name="pv", bufs=1, space="PSUM") as pv_pool,
            tc.tile_pool(name="ep", bufs=3) as ep_pool,
        ):
            batch_tiles = {}
            nm_tiles = {}
            outT_tiles = {}
            pending = []
            PV_LAG = 4

            def load_batch(b, split=False):
                qt = qk_pool.tile([128, S], BF16, tag="qt")
                kt = qk_pool.tile([128, S], BF16, tag="kt")
                v2 = v_pool.tile([128, NK * VW], BF16, tag="v2")
                if split:
                    # first batch: land the data the first QK chunks need
                    # first so the pipeline ramps ~2us earlier
                    nc.sync.dma_start(out=kt[:, 0:QH], in_=KT[b][:, 0:QH])
                    nc.sync.dma_start(out=qt[:, 0:QH], in_=QT[b][:, 0:QH])
                    nc.sync.dma_start(out=kt[:, QH:S], in_=KT[b][:, QH:S])
                    nc.sync.dma_start(out=qt[:, QH:S], in_=QT[b][:, QH:S])
                else:
                    nc.sync.dma_start(out=qt, in_=QT[b])
                    nc.sync.dma_start(out=kt, in_=KT[b])
                nc.sync.dma_start(out=v2, in_=V2[b])
                batch_tiles[b] = (qt, kt, v2)

            def load_nm(b, h):
                t = nm_pool.tile([128, NK * QH], BF16, tag="nm")
                nc.gpsimd.dma_start(out=t, in_=NM8[b, h])
                mb = mb_pool.tile([128, ND * 2 * QH], I16, tag="mb")
                nc.gpsimd.dma_start(out=mb, in_=MB16[b, h])
                nm_tiles[(b, h)] = (t, mb)

            def emit_pv(pt, pair, b, h):
                # lazy outT alloc: first PV of (b,h) claims the buffer; the
                # preceding h's epilogue copy was already emitted by then
                key = (b, h)
                if key not in outT_tiles:
                    outT_tiles[key] = pv_pool.tile([VW, QH], F32, tag="pv", name="outT")
                outT = outT_tiles[key]
                v2 = batch_tiles[b][2]
                for ci, (c, rg) in enumerate(pair):
                    for j in range(2):
                        nc.tensor.matmul(
                            outT[:, j * 512 : (j + 1) * 512],
                            v2[:, c * VW : (c + 1) * VW],
                            pt[:, ci * QH + j * 512 : ci * QH + (j + 1) * 512],
                            start=(c == 0),
                            stop=(c == NK - 1),
                        )
                if pair[1][0] == NK - 1:
                    # last pair of this h: ship unnormalized rows +
                    # denominator row; host divides during unshard
                    otn = ep_pool.tile([D + 1, QH], F32, tag="otn")
                    if (b * NH + h) % 2 == 0:
                        nc.vector.tensor_copy(otn, outT)
                    else:
                        nc.scalar.copy(otn, outT)
                    nc.sync.dma_start(out=OUTT[b, :, h * QH : (h + 1) * QH], in_=otn)
                    del outT_tiles[key]

            load_batch(0, split=True)
            load_nm(0, 0)
            for b in range(BPC):
                if b + 1 < BPC:
                    load_batch(b + 1)
                qt, kt, v2 = batch_tiles[b]
                for h in range(NH):
                    nxt = (b, h + 1) if h + 1 < NH else (b + 1, 0)
                    if nxt[0] < BPC:
                        load_nm(*nxt)
                    nm_all, mb_all = nm_tiles.pop((b, h))
                    # PV trails QK by PV_LAG pairs (carried across h and b
                    # boundaries) so the in-order PE stream never blocks the
                    # next pair's QK on this pair's exp+mask chain
                    for p in range(NP):
                        use_dve = p % 4 == 3
                        use_pool = False  # Pool muls contend SBUF ports with DVE (4x slowdown) -- disabled
                        pair = ((2 * p, 0), (2 * p + 1, 64))
                        pt = pt_pool.tile([128, 2 * QH], BF16, tag="pt")
                        # issue QK interleaved across the pair's row groups
                        # (A0,B0,A1,B1) so both PE streams start together
                        sc_tiles = []
                        for ci in range(2):
                            sc = sc_pool.tile([128, QH], F32, tag="sc")
                            sc_tiles.append(sc)
                        for j in range(2):
                            for ci, (c, rg) in enumerate(pair):
                                q0 = h * QH + j * 512
                                nc.tensor.matmul(
                                    sc_tiles[ci][:, j * 512 : (j + 1) * 512],
                                    kt[rg : rg + 64, c * 128 : (c + 1) * 128],
                                    qt[rg : rg + 64, q0 : q0 + 512],
                                    start=True,
                                    stop=True,
                                )
                        for ci, (c, rg) in enumerate(pair):
                            sc = sc_tiles[ci]
                            dst = pt[:, ci * QH : (ci + 1) * QH]
                            if use_dve:
                                d = p // 4
                                mb_sl = mb_all[
                                    :, (d * 2 + ci) * QH : (d * 2 + ci + 1) * QH
                                ]
                                nc.vector.scalar_tensor_tensor(
                                    dst.bitcast(I16), sc, EXP_A, mb_sl, MULT, ADD
                                )
                            else:
                                nc.scalar.activation(dst, sc, EXP, scale=0.125)
                                if not use_pool:
                                    nc.vector.tensor_mul(
                                        dst, dst, nm_all[:, c * QH : (c + 1) * QH]
                                    )
                        if use_pool:
                            nc.gpsimd.tensor_mul(
                                pt, pt, nm_all[:, 2 * p * QH : 2 * (p + 1) * QH]
                            )
                        pending.append((pt, pair, b, h))
                        if len(pending) > PV_LAG:
                            emit_pv(*pending.pop(0))
            for ent in pending:
                emit_pv(*ent)
    nc.compile()
    _CACHED_NC = nc
    return nc


def prep_inputs(Q, K, V, mask):
    """Host-side layout prep (transposes, duplication for row tiling, bf16)."""
    Q = np.asarray(Q, dtype=np.float32)
    K = np.asarray(K, dtype=np.float32)
    V = np.asarray(V, dtype=np.float32)
    mask = np.asarray(mask)
    QT1 = Q.transpose(0, 2, 1).astype(ml_dtypes.bfloat16)  # [B, D, S]
    KT1 = K.transpose(0, 2, 1).astype(ml_dtypes.bfloat16)
    QT = np.ascontiguousarray(np.concatenate([QT1, QT1], axis=1))  # [B, 128, S]
    KT = np.ascontiguousarray(np.concatenate([KT1, KT1], axis=1))
    # V2[b, p, c, 0:64] = V[b, c*128+p, :]; V2[b, p, c, 64:128] = 1
    V4 = np.ones((B, NK, 128, VW), dtype=ml_dtypes.bfloat16)
    V4[:, :, :, :64] = V.reshape(B, NK, 128, D).astype(ml_dtypes.bfloat16)
    V2 = np.ascontiguousarray(V4.transpose(0, 2, 1, 3).reshape(B, 128, NK * VW))
    # ~mask u8: [b, q, k] -> [b, h, p, (c, q)]
    keepT = (~mask.astype(bool)).transpose(0, 2, 1)  # [B, k, q] bool
    nm5 = keepT.reshape(B, NK, 128, NH, QH)  # [b, c, p, h, q]
    nm = nm5.transpose(0, 3, 2, 1, 4).astype(np.uint8)
    NM8 = np.ascontiguousarray(nm.reshape(B, NH, 128, NK * QH))
    # i16 maskbias for DVE pairs (local p = 4d+3 -> chunks 8d+6, 8d+7)
    sel = nm5[:, [6, 7, 14, 15]]  # [b, (d,ci), p, h, q]
    sel = sel.reshape(B, ND, 2, 128, NH, QH).transpose(0, 4, 3, 1, 2, 5)
    mb = np.where(sel, MB_KEEP, MB_DROP).astype(np.int16)
    MB16 = np.ascontiguousarray(mb.reshape(B, NH, 128, ND * 2 * QH))
    return QT, KT, V2, NM8, MB16


def make_in_maps(Q, K, V, mask):
    QT, KT, V2, NM8, MB16 = prep_inputs(Q, K, V, mask)
    in_maps = []
    for i in range(N_CORES):
        sl = slice(i * BPC, (i + 1) * BPC)
        in_maps.append(
            {"qt": QT[sl], "kt": KT[sl], "v2": V2[sl], "nm8": NM8[sl], "mb16": MB16[sl]}
        )
    return in_maps


def kernel(Q, K, V, mask, dk, **run_kwargs):
    assert int(dk) == D
    nc = build_nc()
    in_maps = make_in_maps(Q, K, V, mask)
    res = run_bass_kernel_spmd(nc, in_maps, list(range(N_CORES)), **run_kwargs)
    out_t = np.concatenate([res.results[i]["out_t"] for i in range(N_CORES)], axis=0)
    out = out_t[:, :D, :] / out_t[:, D : D + 1, :]
    out = np.ascontiguousarray(out.transpose(0, 2, 1)).astype(np.float32)  # [B, S, D]
    if run_kwargs:
        kernel.last_results = res
    return out


# revision 23
# speedup vs baseline: 1.1445x; 1.1445x over previous
"""Masked attention on 8 TRN2 NeuronCores.

Full-input contract: kernel(**inputs) takes the complete Q/K/V/mask/dk and
returns the full [32, 2048, 64] output. Internally shards batch 32 -> 4 per
core (data parallel, no communication).

v4 design (per core: 4 batches, S=2048, D=64), scores computed TRANSPOSED
(S^T[k,q] = K @ Q^T) so exp output P^T is already in the layout the P@V
matmul consumes:
  - QK^T: BF16 matmuls, 2-way PE row tiling (chunk c uses row group c%2;
    Q^T/K^T duplicated into both partition halves), chunk-granular score
    tiles (3 PSUM slots) so QK runs 2-3 chunks ahead of exp.
  - exp: chunk pairs split across engines by local pair index p:
      p%4==3 -> DVE one-instruction exp+mask: scalar_tensor_tensor computes
        round(s*A + mb) -> int16, bitcast bf16 ~ exp(s/8)*~mask, where the
        i16 maskbias mb is 16248 (=127*128-8, Schraudolph bias) on kept
        entries and -32400 (saturates/bitcasts to ~-0.0) on masked ones.
      else -> ScalarE activation Exp (scale=1/8) into bf16, then the mask
        multiply: pair-wide on GpSimd (Pool) for p%4==1, per-chunk on DVE
        otherwise (u8 mask expanded to bf16 by one SWDGE casting DMA per
        (b,h), prefetched one h ahead).
  - P@V transposed: stationary is V chunk [128k, 128] whose cols 64:128 are
    all ones -> outT[0:64,q] = (P@V)^T and outT[64:128,q] = row-sum of P
    (softmax denominator) broadcast at zero extra matmul cycles.
  - epilogue per (b,h): one DVE copy of outT[0:65] -> DMA [b, 65, S].
    Host divides rows 0:64 by row 64 and transposes to [B, S, 64] during
    unshard (the on-device DVE reciprocal at this shape costs 6.5us per
    call; the host divide is 4M flops).
  - qt/kt/v2/maskbias loads prefetch one batch / one h ahead on the SP
    queue; u8 mask rides the SWDGE queue.
"""

import sys

import numpy as np

for _p in ("/opt/trn_rl_repo", "/root/.axon_site/_ro/trn_rl_repo"):
    if _p not in sys.path:
        sys.path.append(_p)

import ml_dtypes

import concourse.bacc as bacc
import concourse.bass as bass
import concourse.mybir as mybir
from concourse.bass_utils import run_bass_kernel_spmd
from concourse.tile import TileContext

N_CORES = 8
B, S, D = 32, 2048, 64
BPC = B // N_CORES  # batches per core
NK = S // 128  # 16 k-chunks
NP = NK // 2  # 8 chunk pairs per (b, h)
NH = 2  # q halves
QH = S // NH  # 1024
ND = 2  # DVE-exp pairs per (b, h)  (p % 4 == 3)
VW = 65  # V stationary width: 64 d cols + 1 ones col (denominator)

F32 = mybir.dt.float32
F32R = mybir.dt.float32r
BF16 = mybir.dt.bfloat16
U8 = mybir.dt.uint8
I16 = mybir.dt.int16
EXP = mybir.ActivationFunctionType.Exp
MULT = mybir.AluOpType.mult
ADD = mybir.AluOpType.add

# int16 Schraudolph exp(s/8): bits = round(s*EXP_A + bias); bias for kept
# entries lives in the maskbias tensor (MB_KEEP), masked entries get MB_DROP
# (saturates to 0x8000 = -0.0 in bf16, or ~1e-35 magnitudes).
EXP_A = 128.0 * (np.log2(np.e) / 8.0)
MB_KEEP = 16248  # 127*128 - 8
# drop bias: s*EXP_A + MB_DROP stays in int16 for |s| < 75 (no wrap) and
# bitcasts to negligible bf16 magnitudes (~2^-120)
MB_DROP = -31000

_CACHED_NC = None


def build_nc():
    global _CACHED_NC
    if _CACHED_NC is not None:
        return _CACHED_NC
    nc = bacc.Bacc("TRN2", target_bir_lowering=False)
    QT = nc.dram_tensor("qt", [BPC, 128, S], BF16, kind="ExternalInput")
    KT = nc.dram_tensor("kt", [BPC, 128, S], BF16, kind="ExternalInput")
    # V2[b, p, c*128:(c+1)*128]: cols 0:64 V chunk c, cols 64:128 all 1.0
    V2 = nc.dram_tensor("v2", [BPC, 128, NK * VW], BF16, kind="ExternalInput")
    # NM8[b, h, p, c*1024+q] = ~mask[b, h*1024+q, c*128+p] as u8
    NM8 = nc.dram_tensor("nm8", [BPC, NH, 128, NK * QH], U8, kind="ExternalInput")
    # MB16[b, h, p, (d, ci, q)]: i16 maskbias for DVE pairs (p_local=4d+3)
    MB16 = nc.dram_tensor("mb16", [BPC, NH, 128, ND * 2 * QH], I16, kind="ExternalInput")
    # rows 0:64 unnormalized (P@V)^T, row 64 = softmax denominator
    OUTT = nc.dram_tensor("out_t", [BPC, D + 1, S], F32, kind="ExternalOutput")

    with TileContext(nc) as tc:
        with (
            tc.tile_pool(name="qk", bufs=3) as qk_pool,
            tc.tile_pool(name="vp", bufs=3) as v_pool,
            tc.tile_pool(name="nm", bufs=3) as nm_pool,
            tc.tile_pool(name="mb", bufs=3) as mb_pool,
            tc.tile_pool(name="pt", bufs=7) as pt_pool,
            tc.tile_pool(name="sc", bufs=3, space="PSUM") as sc_pool,
            tc.tile_pool(name="pv", bufs=1, space="PSUM") as pv_pool,
            tc.tile_pool(name="ep", bufs=3) as ep_pool,
        ):
            batch_tiles = {}
            nm_tiles = {}
            outT_tiles = {}
            pending = []
            PV_LAG = 4

            def load_batch(b, split=False):
                qt = qk_pool.tile([128, S], BF16, tag="qt")
                kt = qk_pool.tile([128, S], BF16, tag="kt")
                v2 = v_pool.tile([128, NK * VW], BF16, tag="v2")
                if split:
                    # first batch: land the data the first QK chunks need
                    # first so the pipeline ramps ~2us earlier
                    nc.sync.dma_start(out=kt[:, 0:QH], in_=KT[b][:, 0:QH])
                    nc.sync.dma_start(out=qt[:, 0:QH], in_=QT[b][:, 0:QH])
                    nc.sync.dma_start(out=kt[:, QH:S], in_=KT[b][:, QH:S])
                    nc.sync.dma_start(out=qt[:, QH:S], in_=QT[b][:, QH:S])
                else:
                    nc.sync.dma_start(out=qt, in_=QT[b])
                    nc.sync.dma_start(out=kt, in_=KT[b])
                nc.sync.dma_start(out=v2, in_=V2[b])
                batch_tiles[b] = (qt, kt, v2)

            def load_nm(b, h):
                t = nm_pool.tile([128, NK * QH], BF16, tag="nm")
                nc.gpsimd.dma_start(out=t, in_=NM8[b, h])
                mb = mb_pool.tile([128, ND * 2 * QH], I16, tag="mb")
                nc.gpsimd.dma_start(out=mb, in_=MB16[b, h])
                nm_tiles[(b, h)] = (t, mb)

            def emit_pv(pt, pair, b, h):
                # lazy outT alloc: first PV of (b,h) claims the buffer; the
                # preceding h's epilogue copy was already emitted by then
                key = (b, h)
                if key not in outT_tiles:
                    outT_tiles[key] = pv_pool.tile([VW, QH], F32, tag="pv", name="outT")
                outT = outT_tiles[key]
                v2 = batch_tiles[b][2]
                for ci, (c, rg) in enumerate(pair):
                    for j in range(2):
                        nc.tensor.matmul(
                            outT[:, j * 512 : (j + 1) * 512],
                            v2[:, c * VW : (c + 1) * VW],
                            pt[:, ci * QH + j * 512 : ci * QH + (j + 1) * 512],
                            start=(c == 0),
                            stop=(c == NK - 1),
                        )
                if pair[1][0] == NK - 1:
                    # last pair of this h: ship unnormalized rows +
                    # denominator row; host divides during unshard
                    otn = ep_pool.tile([D + 1, QH], F32, tag="otn")
                    if (b * NH + h) % 2 == 0:
                        nc.vector.tensor_copy(otn, outT)
                    else:
                        nc.scalar.copy(otn, outT)
                    nc.sync.dma_start(out=OUTT[b, :, h * QH : (h + 1) * QH], in_=otn)
                    del outT_tiles[key]

            load_batch(0, split=True)
            load_nm(0, 0)
            for b in range(BPC):
                if b + 1 < BPC:
                    load_batch(b + 1)
                qt, kt, v2 = batch_tiles[b]
                for h in range(NH):
                    nxt = (b, h + 1) if h + 1 < NH else (b + 1, 0)
                    if nxt[0] < BPC:
                        load_nm(*nxt)
                    nm_all, mb_all = nm_tiles.pop((b, h))
                    # PV trails QK by PV_LAG pairs (carried across h and b
                    # boundaries) so the in-order PE stream never blocks the
                    # next pair's QK on this pair's exp+mask chain
                    for p in range(NP):
                        use_dve = p % 4 == 3
                        use_pool = False  # Pool muls contend SBUF ports with DVE (4x slowdown) -- disabled
                        pair = ((2 * p, 0), (2 * p + 1, 64))
                        pt = pt_pool.tile([128, 2 * QH], BF16, tag="pt")
                        for ci, (c, rg) in enumerate(pair):
                            sc = sc_pool.tile([128, QH], F32, tag="sc")
                            for j in range(2):
                                q0 = h * QH + j * 512
                                nc.tensor.matmul(
                                    sc[:, j * 512 : (j + 1) * 512],
                                    kt[rg : rg + 64, c * 128 : (c + 1) * 128],
                                    qt[rg : rg + 64, q0 : q0 + 512],
                                    start=True,
                                    stop=True,
                                )
                            dst = pt[:, ci * QH : (ci + 1) * QH]
                            if use_dve:
                                d = p // 4
                                mb_sl = mb_all[
                                    :, (d * 2 + ci) * QH : (d * 2 + ci + 1) * QH
                                ]
                                nc.vector.scalar_tensor_tensor(
                                    dst.bitcast(I16), sc, EXP_A, mb_sl, MULT, ADD
                                )
                            else:
                                nc.scalar.activation(dst, sc, EXP, scale=0.125)
                                if not use_pool:
                                    nc.vector.tensor_mul(
                                        dst, dst, nm_all[:, c * QH : (c + 1) * QH]
                                    )
                        if use_pool:
                            nc.gpsimd.tensor_mul(
                                pt, pt, nm_all[:, 2 * p * QH : 2 * (p + 1) * QH]
                            )
                        pending.append((pt, pair, b, h))
                        if len(pending) > PV_LAG:
                            emit_pv(*pending.pop(0))
            for ent in pending:
                emit_pv(*ent)
    nc.compile()
    _CACHED_NC = nc
    return nc


def prep_inputs(Q, K, V, mask):
    """Host-side layout prep (transposes, duplication for row tiling, bf16)."""
    Q = np.asarray(Q, dtype=np.float32)
    K = np.asarray(K, dtype=np.float32)
    V = np.asarray(V, dtype=np.float32)
    mask = np.asarray(mask)
    QT1 = Q.transpose(0, 2, 1).astype(ml_dtypes.bfloat16)  # [B, D, S]
    KT1 = K.transpose(0, 2, 1).astype(ml_dtypes.bfloat16)
    QT = np.ascontiguousarray(np.concatenate([QT1, QT1], axis=1))  # [B, 128, S]
    KT = np.ascontiguousarray(np.concatenate([KT1, KT1], axis=1))
    # V2[b, p, c, 0:64] = V[b, c*128+p, :]; V2[b, p, c, 64:128] = 1
    V4 = np.ones((B, NK, 128, VW), dtype=ml_dtypes.bfloat16)
    V4[:, :, :, :64] = V.reshape(B, NK, 128, D).astype(ml_dtypes.bfloat16)
    V2 = np.ascontiguousarray(V4.transpose(0, 2, 1, 3).reshape(B, 128, NK * VW))
    # ~mask u8: [b, q, k] -> [b, h, p, (c, q)]
    keepT = (~mask.astype(bool)).transpose(0, 2, 1)  # [B, k, q] bool
    nm5 = keepT.reshape(B, NK, 128, NH, QH)  # [b, c, p, h, q]
    nm = nm5.transpose(0, 3, 2, 1, 4).astype(np.uint8)
    NM8 = np.ascontiguousarray(nm.reshape(B, NH, 128, NK * QH))
    # i16 maskbias for DVE pairs (local p = 4d+3 -> chunks 8d+6, 8d+7)
    sel = nm5[:, [6, 7, 14, 15]]  # [b, (d,ci), p, h, q]
    sel = sel.reshape(B, ND, 2, 128, NH, QH).transpose(0, 4, 3, 1, 2, 5)
    mb = np.where(sel, MB_KEEP, MB_DROP).astype(np.int16)
    MB16 = np.ascontiguousarray(mb.reshape(B, NH, 128, ND * 2 * QH))
    return QT, KT, V2, NM8, MB16


def make_in_maps(Q, K, V, mask):
    QT, KT, V2, NM8, MB16 = prep_inputs(Q, K, V, mask)
    in_maps = []
    for i in range(N_CORES):
        sl = slice(i * BPC, (i + 1) * BPC)
        in_maps.append(
            {"qt": QT[sl], "kt": KT[sl], "v2": V2[sl], "nm8": NM8[sl], "mb16": MB16[sl]}
        )
    return in_maps


def kernel(Q, K, V, mask, dk, **run_kwargs):
    assert int(dk) == D
    nc = build_nc()
    in_maps = make_in_maps(Q, K, V, mask)
    res = run_bass_kernel_spmd(nc, in_maps, list(range(N_CORES)), **run_kwargs)
    out_t = np.concatenate([res.results[i]["out_t"] for i in range(N_CORES)], axis=0)
    out = out_t[:, :D, :] / out_t[:, D : D + 1, :]
    out = np.ascontiguousarray(out.transpose(0, 2, 1)).astype(np.float32)  # [B, S, D]
    if run_kwargs:
        kernel.last_results = res
    return out
